# revision 7
# baseline (speedup 1.0000x reference)
"""Trainium2 Bass kernel for nn_CNN_Nested (W2NER-style CNN scorer).

Math (reference):
  head = leaky(wr @ head_w.T + head_b); tail likewise           [B,N,D]
  scores1[b,(h,d),l,k] = sum_{x,y} head[b,l,h,x] U[h,d,x,y] tail[b,k,h,y]
  scores2[b,c,m,n] = h_aug@Wh.T (bcast n) + t_aug@Wt.T (bcast m) + size-emb term
  out = down_w @ (scores1+scores2) + down_b                     [B,OUT,N,N]

down_fc is linear => fold down_w into the constants on the host:
  U'[o,h,x,y] = sum_d down_w[o,h*HD+d] U[h,d,x,y]
  WhD = down_w @ Wh, WtD = down_w @ Wt               (tiny)
  E[o,m,n] = (size_emb @ (down_w@Ws).T)[clip(n-m)+15, o] + down_b[o]
Then per (b, o):
  G[o] = blockdiag(U'[o])^T @ tailT                  [(h,x)=200, N]
  out[b,o] = headT^T @ G[o] + A'[o,m] (x) 1 + 1 (x) B'[o,n] + E[o]
The broadcast adds ride along the group-B matmul: headT_B is augmented with
a ones row (-> B' via gB's B'-row) and the six A'T rows (-> A' via per-pair
indicator rows in gB, built on-device by memset).

All matmul operands are bf16 (PSUM accumulation stays f32); E is added in
bf16 by DVE during PSUM eviction; output is stored bf16 and upcast on host.
Constants ship tightly packed (no 128-row padding for small pieces); the
MLP's four accumulation chains are issued interleaved so weight loads
pipeline behind streams.

Sharding: 8 cores = B(4) x o-half(2x6). No collectives. Full inputs in,
full output out. Hardcoded B=4,N=256,H=768,D=200,NH=5,HD=40,OUT=12.
"""

import os
import numpy as np

B, N, H = 4, 256, 768
D, NH, HD, SZ, OUT = 200, 5, 40, 25, 12
N_POS = 30
OH = OUT // 2          # o's per core
NCORES = 8
GA, GB = 3 * HD, 2 * HD  # 120 / 80: d-rows in partition group A / B
GBX = GB + 7             # group-B rows + ones row + 6 A'T rows

SEG = N + D              # blob1 per-chunk segment [wrt_k | tw_k]
B2A_HW = 6 * D           # b2a: hw_t cols
B2A_SMALL = B2A_HW + 4 * OH   # + whdt_a, whdt_b, wtdt_a, wtdt_b
B2A_COLS_NB = B2A_SMALL       # no-bias variant
B2A_COLS_BIAS = B2A_SMALL + 8  # + hb_a, hb_b, tb_a, tb_b (2 cols each)

_cache = {}
LAST_RESULT = None


def _build_module(has_bias: bool):
    import concourse.bacc as bacc
    import concourse.mybir as mybir
    import concourse.tile as tile
    from concourse.bass import ts
    from contextlib import ExitStack

    dt = mybir.dt
    f32 = dt.float32
    bf = dt.bfloat16
    B2A_COLS = B2A_COLS_BIAS if has_bias else B2A_COLS_NB

    nc = bacc.Bacc("TRN2", target_bir_lowering=False, debug=False,
                   enable_asserts=False, enable_partition_id=False)

    b1_d = nc.dram_tensor("blob1", [128, 6 * SEG], bf, kind="ExternalInput").ap()
    b2a_d = nc.dram_tensor("b2a", [128, B2A_COLS], bf, kind="ExternalInput").ap()
    bd_d = nc.dram_tensor("bd", [GA, 1200], bf, kind="ExternalInput").ap()
    e_d = nc.dram_tensor("e_pack", [128, OH * 512], bf,
                         kind="ExternalInput").ap()
    out_d = nc.dram_tensor("out", [3, 128, 1024], bf, kind="ExternalOutput").ap()

    with tile.TileContext(nc) as tc, ExitStack() as ctx:
        sb = ctx.enter_context(tc.tile_pool(name="sb", bufs=1))
        tmp_pool = ctx.enter_context(tc.tile_pool(name="tmp", bufs=2))
        pa_stack = ExitStack()
        pa = pa_stack.enter_context(tc.tile_pool(name="pa", bufs=2,
                                                 space="PSUM"))

        # ---- input DMAs first: sync and scalar are the HW-DGE engines ------
        b10_s = sb.tile([128, SEG], bf, tag="b10", name="b10")
        nc.sync.dma_start(b10_s[:], b1_d[:, 0:SEG])
        b11_s = sb.tile([128, 2 * SEG], bf, tag="b11", name="b11")
        nc.scalar.dma_start(b11_s[:], b1_d[:, SEG:3 * SEG])
        b12_s = sb.tile([128, 3 * SEG], bf, tag="b12", name="b12")
        nc.sync.dma_start(b12_s[:], b1_d[:, 3 * SEG:6 * SEG])
        b2a_s = sb.tile([128, B2A_COLS], bf, tag="b2a", name="b2a")
        nc.scalar.dma_start(b2a_s[:], b2a_d[:, :])
        bd_s = sb.tile([GA, 1200], bf, tag="bd", name="bd")
        nc.sync.dma_start(bd_s[:], bd_d[:, :])
        # e_pack in 3 bank-sized pieces so each out_bank only waits its part
        e_s = []
        for p in range(3):
            e_t = sb.tile([128, 1024], bf, tag=f"es{p}", name=f"es{p}")
            nc.scalar.dma_start(e_t[:], e_d[:, ts(p, 1024)])
            e_s.append(e_t)

        if os.environ.get("KERNEL_WARMUP"):
            scratch = sb.tile([128, 512], bf, tag="warm", name="warm")
            nc.vector.memset(scratch[:], 0.0)
            for _ in range(2):
                wps = pa.tile([128, 512], f32, tag="wps", name="wps", bufs=1)
                nc.tensor.matmul(wps[:], scratch[0:128, 0:128], scratch[:],
                                 start=True, stop=True)

        def _seg(k):
            if k == 0:
                return b10_s, 0
            if k < 3:
                return b11_s, (k - 1) * SEG
            return b12_s, (k - 3) * SEG

        def wrT(k):
            t, c = _seg(k)
            return t[:, c:c + N]

        def tw_slice(k, off, sz):
            t, c = _seg(k)
            return t[:, c + N + off:c + N + off + sz]

        def hw_slice(k, off, sz):
            c = k * D + off
            return b2a_s[0:128, c:c + sz]

        def small(idx, rows):
            c = B2A_HW + idx * OH
            return b2a_s[0:rows, c:c + OH]

        def bias_col(idx, rows):
            c = B2A_SMALL + idx * 2
            return b2a_s[0:rows, c:c + 2]

        # ---- headT/tailT = leaky(w @ wr^T + b), [d, l] layout ---------------
        # group A rows d in [0,120); group B rows d in [120,200), then a ones
        # row at 80 and the six A'T rows at 81..86 (written later).
        headT_A = sb.tile([GA, N], bf, tag="hTA", name="hTA")
        headT_B = sb.tile([GBX, N], bf, tag="hTB", name="hTB")
        tailT_A = sb.tile([GA, N], bf, tag="tTA", name="tTA")
        tailT_B = sb.tile([GB + 1, N], bf, tag="tTB", name="tTB")

        # four interleaved accumulation chains: weight loads pipeline behind
        # the previous chain's stream.
        chains = [
            ("tA", tw_slice, 0, GA, tailT_A[:], 2),
            ("tB", tw_slice, GA, GB, tailT_B[0:GB, :], 3),
            ("hA", hw_slice, 0, GA, headT_A[:], 0),
            ("hB", hw_slice, GA, GB, headT_B[0:GB, :], 1),
        ]
        ps_mlp = {}
        for tag, _, _, sz, _, _ in chains:
            ps_mlp[tag] = pa.tile([sz, N], f32, tag=f"pm{tag}",
                                  name=f"pm{tag}", bufs=1)
        for hk in range(6):
            for tag, wsl, off, sz, _, _ in chains:
                nc.tensor.matmul(ps_mlp[tag][:], wsl(hk, off, sz), wrT(hk),
                                 start=(hk == 0), stop=(hk == 5))
        for tag, _, off, sz, dst, bidx in chains:
            ps = ps_mlp[tag]
            if has_bias:
                tsc = tmp_pool.tile([sz, N], f32, tag="tsc", name="tsc")
                tln = tmp_pool.tile([sz, N], f32, tag="tln", name="tln")
                bias = bias_col(bidx, sz)
                nc.scalar.activation(tln[:], ps[:],
                                     mybir.ActivationFunctionType.Copy,
                                     bias=bias[:, 0:1])
                nc.scalar.activation(tsc[:], ps[:],
                                     mybir.ActivationFunctionType.Copy,
                                     bias=bias[:, 1:2], scale=0.01)
                nc.vector.tensor_max(dst, tln[:], tsc[:])
            else:
                tsc = tmp_pool.tile([sz, N], f32, tag="tsc", name="tsc")
                nc.vector.tensor_scalar_mul(tsc[:], ps[:], 0.01)
                nc.vector.tensor_max(dst, ps[:], tsc[:])

        # ones + indicator rows: memset at partition base 0, then SBUF->SBUF
        # DMA into the unaligned partitions (engines need 32-aligned bases).
        ones_s = sb.tile([1, N], bf, tag="ones", name="ones")
        nc.vector.memset(ones_s[:], 1.0)
        # ind[j, c] = 1{c//256 == j}: memset 1 then zero where cb - j != 0
        ind_s = sb.tile([6, 3 * 512], bf, tag="ind", name="ind")
        nc.vector.memset(ind_s[:], 1.0)
        ind3 = ind_s[:].rearrange("p (cb c) -> p cb c", cb=6)
        nc.gpsimd.affine_select(ind3, ind3, [[1, 6], [0, N]],
                                mybir.AluOpType.is_equal, 0.0,
                                base=0, channel_multiplier=-1)
        nc.sync.dma_start(tailT_B[GB:GB + 1, :], ones_s[:])
        nc.scalar.dma_start(headT_B[GB:GB + 1, :], ones_s[:])

        # ---- B'T / A'T projections [OH, N] ----------------------------------
        def proj(ia, ib, srcA, srcB, tagc):
            ps = pa.tile([OH, N], f32, tag="pap", name="pap", bufs=2)
            nc.tensor.matmul(ps[:], small(ia, GA), srcA[:], start=True,
                             stop=False)
            nc.tensor.matmul(ps[:], small(ib, GB + 1), srcB[0:GB + 1, :],
                             start=False, stop=True)
            flat = sb.tile([OH, N], bf, tag=f"{tagc}f", name=f"{tagc}f")
            nc.vector.tensor_copy(flat[:], ps[:])
            return flat

        Bpf = proj(2, 3, tailT_A, tailT_B, "Bp")
        Apf = proj(0, 1, headT_A, headT_B, "Ap")
        nc.sync.dma_start(headT_B[GB + 1:GBX, :], Apf[:, :])

        pa_stack.close()
        pg = ctx.enter_context(tc.tile_pool(name="pg", bufs=2,
                                            space="PSUM"))
        po = ctx.enter_context(tc.tile_pool(name="po", bufs=3, space="PSUM"))

        gAt, gBt = [], []

        def g_build(p):
            gA = sb.tile([GA, 512], bf, tag=f"gA{p}", name=f"gA{p}")
            gB = sb.tile([GBX, 512], bf, tag=f"gB{p}", name=f"gB{p}")
            for half in range(2):
                j = 2 * p + half
                psa = pg.tile([GA, N], f32, tag="psga", name="psga")
                nc.tensor.matmul(psa[:], bd_s[0:GA, ts(j, GA)],
                                 tailT_A[:], start=True, stop=True)
                nc.vector.tensor_copy(gA[:, ts(half, N)], psa[:])
                psb = pg.tile([GB, N], f32, tag="psgb", name="psgb")
                nc.tensor.matmul(psb[:], bd_s[0:GB, 720 + j * GB:720 + (j + 1) * GB],
                                 tailT_B[0:GB, :], start=True, stop=True)
                nc.vector.tensor_copy(gB[0:GB, ts(half, N)], psb[:])
            # row 80: [B'T[o0] | B'T[o1]]  (pairs with headT_B's ones row)
            nc.sync.dma_start(gB[GB:GB + 1, :], Bpf[2 * p:2 * p + 2, :])
            # rows 81..86: indicator rows (pair with headT_B's A'T rows)
            nc.scalar.dma_start(gB[GB + 1:GBX, :], ind_s[:, ts(p, 512)])
            gAt.append(gA)
            gBt.append(gB)

        def out_bank(p):
            out_s = sb.tile([128, 1024], bf, tag=f"os{p}", name=f"os{p}")
            for lt in range(2):
                ob = po.tile([128, 512], f32, tag="ob", name="ob")
                nc.tensor.matmul(ob[:], headT_A[:, ts(lt, 128)], gAt[p][:],
                                 start=True, stop=False)
                nc.tensor.matmul(ob[:], headT_B[:, ts(lt, 128)], gBt[p][:],
                                 start=False, stop=True)
                nc.vector.tensor_add(out_s[:, ts(lt, 512)], ob[:],
                                     e_s[p][:, ts(lt, 512)])
            eng = nc.scalar if p == 1 else nc.sync
            eng.dma_start(out_d[p], out_s[:])

        g_build(0)
        g_build(1)
        out_bank(0)
        g_build(2)
        out_bank(1)
        out_bank(2)

    nc.compile()
    return nc


def _get_module(has_bias: bool):
    key = ("mod", has_bias)
    if key not in _cache:
        _cache[key] = _build_module(has_bias)
    return _cache[key]


def _host_pack(head_w, head_b, tail_w, tail_b, U_mh, size_emb, W, down_w,
               down_b):
    """Fold down_w into the constants; build per-o-half bf16 blob layouts."""
    from ml_dtypes import bfloat16
    f64 = np.float64
    d1 = D + 1
    Wh, Wt, Ws = W[:, :d1], W[:, d1:2 * d1], W[:, 2 * d1:]
    WhD = (down_w.astype(f64) @ Wh.astype(f64)).astype(np.float32)   # [OUT,D+1]
    WtD = (down_w.astype(f64) @ Wt.astype(f64)).astype(np.float32)
    WsD = (down_w.astype(f64) @ Ws.astype(f64)).astype(np.float32)   # [OUT,SZ]
    ct = (size_emb.astype(f64) @ WsD.T.astype(f64)).astype(np.float32)
    dw_r = down_w.reshape(OUT, NH, HD)
    Up = np.einsum('ohd,hdxy->ohxy', dw_r.astype(f64),
                   U_mh.astype(f64)).astype(np.float32)              # [OUT,NH,HD,HD]

    idx = np.arange(N)
    span = np.clip(idx[None, :] - idx[:, None], -N_POS // 2,
                   N_POS // 2 - 1) + N_POS // 2
    E = ct[span].transpose(2, 0, 1) + down_b[:, None, None]          # [OUT,N,N]

    has_bias = bool(np.any(head_b) or np.any(tail_b))
    B2A_COLS = B2A_COLS_BIAS if has_bias else B2A_COLS_NB

    def pack_w(wmat):  # [D,H] -> [128, 6*200]
        return np.ascontiguousarray(
            wmat.T.reshape(6, 128, D).transpose(1, 0, 2).reshape(128, 6 * D))

    twp = pack_w(tail_w)
    hwp = pack_w(head_w)

    blobs_oh = []
    for oh in range(2):
        osl = slice(oh * OH, (oh + 1) * OH)

        # blob1: six [wrt_k | tw_k] segments; wrt filled per-batch later
        blob1 = np.zeros((128, 6 * SEG), np.float32)
        for k in range(6):
            blob1[:, k * SEG + N:(k + 1) * SEG] = twp[:, k * D:(k + 1) * D]

        # b2a: hw_t + the four small projection blocks (+ biases)
        b2a = np.zeros((128, B2A_COLS), np.float32)
        b2a[:, 0:B2A_HW] = hwp
        b2a[0:GA, B2A_HW:B2A_HW + OH] = WhD[osl, 0:GA].T
        b2a[0:GB + 1, B2A_HW + OH:B2A_HW + 2 * OH] = np.concatenate(
            [WhD[osl, GA:D].T, WhD[osl, D:D + 1].T], axis=0)
        b2a[0:GA, B2A_HW + 2 * OH:B2A_HW + 3 * OH] = WtD[osl, 0:GA].T
        b2a[0:GB + 1, B2A_HW + 3 * OH:B2A_HW + 4 * OH] = np.concatenate(
            [WtD[osl, GA:D].T, WtD[osl, D:D + 1].T], axis=0)
        if has_bias:
            b2a[0:GA, B2A_SMALL:B2A_SMALL + 2] = np.stack(
                [head_b[0:GA], 0.01 * head_b[0:GA]], axis=1)
            b2a[0:GB, B2A_SMALL + 2:B2A_SMALL + 4] = np.stack(
                [head_b[GA:D], 0.01 * head_b[GA:D]], axis=1)
            b2a[0:GA, B2A_SMALL + 4:B2A_SMALL + 6] = np.stack(
                [tail_b[0:GA], 0.01 * tail_b[0:GA]], axis=1)
            b2a[0:GB, B2A_SMALL + 6:B2A_SMALL + 8] = np.stack(
                [tail_b[GA:D], 0.01 * tail_b[GA:D]], axis=1)

        # bd: blockdiag U' for groups A (cols 0:720) and B (cols 720:1200)
        UpS = Up[osl]
        bd = np.zeros((GA, 1200), np.float32)
        bd_a = np.zeros((OH, GA, GA), np.float32)
        bd_b = np.zeros((OH, GB, GB), np.float32)
        for h in range(3):
            bd_a[:, h * HD:(h + 1) * HD, h * HD:(h + 1) * HD] = \
                UpS[:, h].transpose(0, 2, 1)
        for h in range(2):
            bd_b[:, h * HD:(h + 1) * HD, h * HD:(h + 1) * HD] = \
                UpS[:, 3 + h].transpose(0, 2, 1)
        bd[:, 0:720] = bd_a.transpose(1, 0, 2).reshape(GA, OH * GA)
        bd[0:GB, 720:1200] = bd_b.transpose(1, 0, 2).reshape(GB, OH * GB)

        e_pack = np.zeros((128, OH * 512), np.float32)
        for p in range(OH // 2):
            for lt in range(2):
                o0 = oh * OH + 2 * p
                c0 = (2 * p + lt) * 512
                e_pack[:, c0:c0 + N] = E[o0, lt * 128:(lt + 1) * 128, :]
                e_pack[:, c0 + N:c0 + 512] = E[o0 + 1,
                                               lt * 128:(lt + 1) * 128, :]

        blobs_oh.append((blob1.astype(bfloat16), b2a.astype(bfloat16),
                         bd.astype(bfloat16), e_pack.astype(bfloat16)))
    return blobs_oh, has_bias


def _ensure_axon():
    """If a host-side jax.config pinned the cpu platform (e.g. to run the
    reference), switch back to the axon/neuron backend for the device run."""
    import jax
    try:
        if any(getattr(d, 'platform', '') == 'axon' for d in jax.devices()):
            return
    except Exception:
        pass
    try:
        import jax.extend
        jax.config.update('jax_platforms', 'axon')
        jax.extend.backend.clear_backends()
    except Exception:
        pass


def _make_in_maps(word_reps, blobs_oh):
    from ml_dtypes import bfloat16
    wrt_b = []
    for b in range(B):
        wrt = word_reps[b].T.reshape(6, 128, N).transpose(1, 0, 2) \
            .reshape(128, 6 * N)
        wrt_b.append(wrt.astype(bfloat16))
    in_maps = []
    for core in range(NCORES):
        b, oh = core // 2, core % 2
        b1, b2a, bd, ep = blobs_oh[oh]
        b1 = b1.copy()
        for k in range(6):
            b1[:, k * SEG:k * SEG + N] = wrt_b[b][:, k * N:(k + 1) * N]
        in_maps.append(dict(blob1=b1, b2a=b2a, bd=bd, e_pack=ep))
    return in_maps


def kernel(word_reps, cls_embeding=None, pieces_index=None, loss_mask=None,
           head_w=None, head_b=None, tail_w=None, tail_b=None, U_mh=None,
           size_emb=None, W=None, down_w=None, down_b=None, **_unused):
    global LAST_RESULT
    from concourse import bass_utils

    word_reps = np.asarray(word_reps, np.float32)
    args = [np.asarray(a, np.float32) for a in
            (head_w, head_b, tail_w, tail_b, U_mh, size_emb, W, down_w,
             down_b)]
    blobs_oh, has_bias = _host_pack(*args)

    nc = _get_module(has_bias)

    in_maps = _make_in_maps(word_reps, blobs_oh)
    _ensure_axon()

    trace = bool(os.environ.get("KERNEL_TRACE"))
    res = bass_utils.run_bass_kernel_spmd(nc, in_maps, list(range(NCORES)),
                                          trace=trace)
    LAST_RESULT = res

    out = np.empty((B, OUT, N, N), np.float32)
    for core in range(NCORES):
        b, oh = core // 2, core % 2
        dev = np.asarray(res.results[core]["out"], dtype=np.float32)
        # dev[p, q, lt*512 + half*256 + n] -> out[o=2p+half, m=lt*128+q, n]
        arr = dev.reshape(3, 128, 2, 2, 256).transpose(0, 3, 2, 1, 4) \
            .reshape(OH, N, N)
        out[b, oh * OH:(oh + 1) * OH] = arr
    return out
